# revision 1
# baseline (speedup 1.0000x reference)
"""Trainium2 Bass kernel for AdultConnectomeNetwork (gnn_message_passing).

Reference computation:
    A = scatter(rows, cols, adj_vals)   # [N, N] dense from COO, dups sum
    W = scatter(rows, cols, w_vals)     # [N, N]
    M = A @ W
    for _ in range(4): x = M @ x + bias[None, :]

Algebraic rewrite (hybrid): layers 1-2 applied directly with M, layers
3-4 fused through M^2:
    X2 = M (M X + 1 b^T) + 1 b^T
    X4 = M^2 X2 + s b^T,   s = M @ 1 + 1
since x <- Mx + 1 b^T twice gives M^2 x + ((M + I) 1) b^T.

Precision: fp8 e4m3 matmuls in DoubleRow perf mode (2 contraction k-tiles
per instruction at 0.5 cyc/row). The positive matrices (A, W, M, M^2) are
stored single-fp8; the signed X, X1, X2 are split hi+lo fp8. Power-of-2
scales keep fp8 in range (M^2 stored as SQ*M^2, X2 as SX2*X2); the
product scale divides out on the final PSUM->SBUF copy and pre-multiplies
the bias row. End-to-end rel err ~8.4e-3 (vs 2e-2 gate).

Distribution (8 NeuronCores): column-shard x and the M/M^2 column blocks,
with everything pipelined at 512-column "quarter" granularity so the DMA
fabric (the binding resource: ~70us of unavoidable traffic at 360 GB/s)
never idles:
  stage h: P1 chunk h (McT = WcT @ AT, AT streamed), fp8 split, stage the
      quarter out, AllGather it (one broadcast per quarter), load the
      gathered MT quarter back as a single [128,16,512] tile; meanwhile
      the PE transposes the previous chunk into Mc and runs the previous
      quarter's consumers: L1 m-tiles (X1 = M X + 1 b^T) and row sums.
  after P1: P2 quarters (M2cT from Mc x MT), each split/staged/gathered/
      loaded the same way; L2 = M X1 + 1 b^T runs under the AG2 traffic;
      A2 = M^2 X2 + s b^T consumes the M2T quarters as they land, output
      unscaled on the PSUM->SBUF copy (bf16 out, cast to f32 on host).
Bias enters each PSUM group as a K=1 bf16 matmul (lhsT = ones/s row).
Scheduling notes: long clean PE bursts (the cost-model clock ramps only
under continuous execution); every matmul group depends on exactly one
quarter tile so the engine's 4-deep unsatisfied-wait queue never clogs;
AG stand-in (sim_single_core) is one stride-0-source broadcast copy per
quarter (same bytes/descriptors as 8 rank-slice copies).
Host: dense scatter of the COO (np.bincount), fp8 casts/splits, shard,
run SPMD, concat cols.
"""

import numpy as np

import concourse.bass as bass
import concourse.mybir as mybir
from concourse import bacc, tile
from concourse.bass_utils import run_bass_kernel_spmd

N = 2048
NNZ = 131072
LAYERS = 4
N_CORES = 8
NB = N // N_CORES          # 256 columns of x per core
KT = N // 128              # 16 k-tiles
KP = KT // 2               # 8 DoubleRow k-pairs
RB = 8                     # contraction pairs per matmul group
NQ = 4                     # 512-column quarters
Q = N // NQ                # 512

SQ = 2.0 ** -3             # scale on stored M^2 (max|M^2| ~1119 -> ~140)
SX2 = 2.0 ** -14           # scale on stored X2   (max|X2*SX2| ~6.7)
UN = 1.0 / (SQ * SX2)      # PSUM(A2) -> true X4
WARM = 0                  # warm-keeping transposes

DEFAULT_DT = "fp8"
_DT = {"f32r": mybir.dt.float32r, "f32": mybir.dt.float32,
       "bf16": mybir.dt.bfloat16, "fp8": mybir.dt.float8e4}
DR = mybir.MatmulPerfMode.DoubleRow
F32 = mybir.dt.float32
BF16 = mybir.dt.bfloat16
FP8 = mybir.dt.float8e4
COPY = mybir.ActivationFunctionType.Copy


def build_nc(iters: int = 1, sim_single_core: bool = False, dt: str = DEFAULT_DT) -> bacc.Bacc:
    """sim_single_core: replace each AllGather with a broadcast copy so the
    graph is collective-free (runnable under TimelineSim) while keeping the
    same dependency structure + data volume (bandwidth-honest stand-in).
    That variant is NOT functionally correct."""
    nc = bacc.Bacc("TRN2", target_bir_lowering=False, num_devices=N_CORES)

    at_d = nc.dram_tensor("at", [N, N], FP8, kind="ExternalInput")
    # pre-arranged on host to SBUF partition-major layouts (contiguous loads)
    wc_d = nc.dram_tensor("wc", [128, KT * NB], FP8, kind="ExternalInput")
    xhl_d = nc.dram_tensor("xhl", [128, KT * 2 * NB], FP8, kind="ExternalInput")
    bias_d = nc.dram_tensor("biasc", [1, NB], BF16, kind="ExternalInput")
    ones_d = nc.dram_tensor("onesr", [1, 128], BF16, kind="ExternalInput")
    # s1r = (A @ (W @ 1) + 1) * SQ * SX2, the bias row for A2 (host-side
    # O(nnz) marshalling from the COO, like the dense scatter itself)
    s1r_d = nc.dram_tensor("s1r", [1, N], BF16, kind="ExternalInput")
    out_d = nc.dram_tensor("out", [N, NB], BF16, kind="ExternalOutput")

    with tile.TileContext(nc) as tc:
        with (
            tc.tile_pool(name="const", bufs=1) as constp,
            tc.tile_pool(name="x", bufs=1) as xp,
            tc.tile_pool(name="dram", bufs=1, space="DRAM") as dram,
        ):
            bias_sb = constp.tile([1, NB], BF16, tag="bias")
            onesr = constp.tile([1, 128], BF16, tag="onesr")
            s1a2 = constp.tile([1, N], BF16, tag="s1a2")

            for it in range(iters):
                # per-iter DRAM AG staging (Shared allows only one writer)
                ag_as = "Local" if sim_single_core else "Shared"
                mct_q = [dram.tile([NB, Q], FP8, name=f"mct{q}_{it}")
                         for q in range(NQ)]
                mtf_q = [dram.tile([N, Q], FP8, name=f"mtf{q}_{it}",
                                   addr_space=ag_as) for q in range(NQ)]
                m2ct_q = [dram.tile([NB, Q], FP8, name=f"m2ct{q}_{it}")
                          for q in range(NQ)]
                m2tf_q = [dram.tile([N, Q], FP8, name=f"m2tf{q}_{it}",
                                    addr_space=ag_as) for q in range(NQ)]

                # per-iter SBUF tiles (same tags -> same space across iters)
                wc_sb = xp.tile([128, KT * NB], FP8, tag="wc", name=f"wc_{it}")
                wc3 = wc_sb[:, :].rearrange("p (k c) -> p k c", k=KT)
                xhl_sb = xp.tile([128, KT * 2 * NB], FP8, tag="xhl", name=f"xhl_{it}")
                xhl4 = xhl_sb[:, :].rearrange("p (k h c) -> p k h c", k=KT, h=2)
                mc_sb = xp.tile([128, KT * NB], FP8, tag="mc", name=f"mc_{it}")
                mc3 = mc_sb[:, :].rearrange("p (k c) -> p k c", k=KT)
                mct_sb = xp.tile([128, 2 * N], FP8, tag="mct", name=f"mct_{it}")
                mct3 = mct_sb[:, :].rearrange("p (m j) -> p m j", m=2)
                m2ct_sb = xp.tile([128, 2 * N], FP8, tag="m2ct", name=f"m2ct_{it}")
                m2ct3 = m2ct_sb[:, :].rearrange("p (m j) -> p m j", m=2)
                x1h_sb = xp.tile([128, KT * NB], FP8, tag="x1h", name=f"x1h_{it}")
                x1h3 = x1h_sb[:, :].rearrange("p (k c) -> p k c", k=KT)
                x1l_sb = xp.tile([128, KT * NB], FP8, tag="x1l", name=f"x1l_{it}")
                x1l3 = x1l_sb[:, :].rearrange("p (k c) -> p k c", k=KT)
                x2h_sb = xp.tile([128, KT * NB], FP8, tag="x2h", name=f"x2h_{it}")
                x2h3 = x2h_sb[:, :].rearrange("p (k c) -> p k c", k=KT)
                x2l_sb = xp.tile([128, KT * NB], FP8, tag="x2l", name=f"x2l_{it}")
                x2l3 = x2l_sb[:, :].rearrange("p (k c) -> p k c", k=KT)
                # gathered MT / M2T quarters: one [128, KT, 512] tile each
                mtq = [xp.tile([128, KT * Q], FP8, tag=f"mtq{q}",
                               name=f"mtq{q}_{it}") for q in range(NQ)]
                mtq3 = [t[:, :].rearrange("p (k j) -> p k j", k=KT) for t in mtq]
                m2tq = [xp.tile([128, KT * Q], FP8, tag=f"m2tq{q}",
                                name=f"m2tq{q}_{it}") for q in range(NQ)]
                m2tq3 = [t[:, :].rearrange("p (k j) -> p k j", k=KT) for t in m2tq]

                def allgather(src_d, dst_d):
                    if sim_single_core:
                        # one broadcast: the 8 rank-slices as a stride-0
                        # source repeat (same bytes/descriptors as 8 copies)
                        sap = src_d[:, :]
                        rep = bass.AP(sap.tensor, sap.offset,
                                      [[0, N_CORES]] + list(sap.ap))
                        nc.gpsimd.dma_start(
                            out=dst_d[:, :].rearrange("(r p) j -> r p j",
                                                      r=N_CORES),
                            in_=rep)
                    else:
                        nc.gpsimd.collective_compute(
                            "AllGather", mybir.AluOpType.bypass,
                            replica_groups=[list(range(N_CORES))],
                            ins=[src_d.opt()], outs=[dst_d.opt()],
                        )

                def load_q(dst3, src_d):
                    # whole gathered quarter in one DMA (2048-row source)
                    nc.gpsimd.dma_start(
                        out=dst3[:, :, :],
                        in_=src_d[:, :].rearrange("(k p) j -> p k j", p=128),
                    )

                # bias matmul closes each accumulation group (K=1, bf16)
                def bias_mm(ps, row):
                    nc.tensor.matmul(ps[:, 0:NB], row, bias_sb[0:1, :],
                                     start=False, stop=True,
                                     skip_group_check=True)

                def split_to(h3, l3, scale):
                    # X1/X2 columns are dominated by M's top eigenvector, so
                    # elementwise fp8 noise is sqrt(N)-suppressed after the
                    # next contraction: a single fp8 plane suffices (no lo)
                    def close(ps, m):
                        if m % 2 == 0:
                            if scale == 1.0:
                                nc.scalar.copy(h3[:, m, :], ps[:, 0:NB])
                            else:
                                nc.scalar.mul(h3[:, m, :], ps[:, 0:NB], scale)
                        else:
                            nc.vector.tensor_scalar_mul(h3[:, m, :], ps[:, 0:NB],
                                                        scale)
                    return close

                xhi = xhl4[:, :, 0, :]
                xlo = xhl4[:, :, 1, :]
                l1_close = split_to(x1h3, x1l3, 1.0)
                l2_close = split_to(x2h3, x2l3, SX2)

                with (
                    tc.tile_pool(name="at", bufs=1) as atp,
                    tc.tile_pool(name="ps1", bufs=2, space="PSUM") as ps1p,
                    tc.tile_pool(name="psmc", bufs=2, space="PSUM") as psmcp,
                    tc.tile_pool(name="ps2", bufs=2, space="PSUM") as ps2p,
                    tc.tile_pool(name="psl1", bufs=2, space="PSUM") as psl1p,
                ):
                    if it == 0:
                        nc.scalar.dma_start(out=wc_sb[:, 0:512], in_=wc_d[:, 0:512])
                        nc.scalar.dma_start(out=wc_sb[:, 512:], in_=wc_d[:, 512:])
                        nc.scalar.dma_start(out=bias_sb[:, :], in_=bias_d[:, :])
                        nc.scalar.dma_start(out=onesr[:, :], in_=ones_d[:, :])
                        nc.scalar.dma_start(out=s1a2[:, :], in_=s1r_d[:, :])
                    else:
                        nc.scalar.dma_start(out=wc_sb[:, :], in_=wc_d[:, :])
                    nc.scalar.dma_start(out=xhl_sb[:, :], in_=xhl_d[:, :])

                    at_t = [atp.tile([128, KT, Q], FP8, tag=f"at{h}",
                                     name=f"at{h}_{it}") for h in range(NQ)]
                    for h in range(NQ):
                        src = at_d[:, Q * h:Q * (h + 1)] \
                            .rearrange("(k p) j -> p k j", p=128)
                        if h == 0:
                            nc.sync.dma_start(out=at_t[0][:, 0:4, :], in_=src[:, 0:4, :])
                            nc.sync.dma_start(out=at_t[0][:, 4:, :], in_=src[:, 4:, :])
                        else:
                            nc.sync.dma_start(out=at_t[h][:, :, :], in_=src)

                    def p1_chunk(h):
                        for mi in range(2):
                            ps = ps1p.tile([128, 512], F32, tag="ps1", name="ps1")
                            for kp in range(KP):
                                nc.tensor.matmul(
                                    ps[:, :],
                                    wc3[:, 2 * kp:2 * kp + 2, 128 * mi:128 * (mi + 1)],
                                    at_t[h][:, 2 * kp:2 * kp + 2, :],
                                    start=(kp == 0), stop=(kp == KP - 1),
                                    perf_mode=DR, skip_group_check=True,
                                )
                            dst = mct3[:, mi, Q * h:Q * (h + 1)]
                            if mi == 0:
                                nc.scalar.copy(dst, ps[:, :])
                            else:
                                nc.vector.tensor_copy(dst, ps[:, :])
                        nc.scalar.dma_start(
                            out=mct_q[h][:, :].rearrange("(m p) j -> p m j", p=128),
                            in_=mct3[:, :, Q * h:Q * (h + 1)],
                        )
                        allgather(mct_q[h], mtf_q[h])
                        load_q(mtq3[h], mtf_q[h])

                    def mc_batch(h):
                        # Mc[i, c] = sum_k AT[k, i] Wc[k, c]: the same chunk
                        # tiles in the other orientation (AT stationary)
                        for jb in range(4):
                            ps = psmcp.tile([128, 512], F32, tag="psmc", name="psmc")
                            for kp in range(KP):
                                nc.tensor.matmul(
                                    ps[:, 0:NB],
                                    at_t[h][:, 2 * kp:2 * kp + 2,
                                            128 * jb:128 * (jb + 1)],
                                    wc3[:, 2 * kp:2 * kp + 2, :],
                                    start=(kp == 0), stop=(kp == KP - 1),
                                    perf_mode=DR, skip_group_check=True,
                                )
                            if jb % 2 == 0:
                                nc.scalar.copy(mc3[:, 4 * h + jb, :], ps[:, 0:NB])
                            else:
                                nc.vector.tensor_copy(mc3[:, 4 * h + jb, :],
                                                      ps[:, 0:NB])

                    def layer_m(lhs_at, rh3, rl3, bias_row, close, pool, ms):
                        for m in ms:
                            ps = pool.tile([128, 512], F32, tag="psl", name="psl")
                            for r in range(RB):
                                for rhs3 in ([rh3] if rl3 is None else [rh3, rl3]):
                                    nc.tensor.matmul(
                                        ps[:, 0:NB],
                                        lhs_at(r, m),
                                        rhs3[:, 2 * r:2 * r + 2, :],
                                        start=(r == 0 and rhs3 is rh3), stop=False,
                                        perf_mode=DR, skip_group_check=True,
                                    )
                            bias_mm(ps, bias_row(m))
                            close(ps, m)

                    def mt_at(r, m):
                        return mtq3[m // 4][:, 2 * r:2 * r + 2,
                                           128 * (m % 4):128 * (m % 4 + 1)]

                    def m2t_at(r, m):
                        return m2tq3[m // 4][:, 2 * r:2 * r + 2,
                                             128 * (m % 4):128 * (m % 4 + 1)]

                    def p2_q(q):
                        for mi in range(2):
                            ps = ps2p.tile([128, 512], F32, tag="ps2", name="ps2")
                            for r in range(RB):
                                nc.tensor.matmul(
                                    ps[:, :],
                                    mc3[:, 2 * r:2 * r + 2, 128 * mi:128 * (mi + 1)],
                                    mtq3[q][:, 2 * r:2 * r + 2, :],
                                    start=(r == 0), stop=(r == RB - 1),
                                    perf_mode=DR, skip_group_check=True,
                                )
                            dst = m2ct3[:, mi, Q * q:Q * (q + 1)]
                            if mi == 0:
                                nc.scalar.mul(dst, ps[:, :], SQ)
                            else:
                                nc.vector.tensor_scalar_mul(dst, ps[:, :], SQ)
                        nc.sync.dma_start(
                            out=m2ct_q[q][:, :].rearrange("(m p) j -> p m j", p=128),
                            in_=m2ct3[:, :, Q * q:Q * (q + 1)],
                        )
                        allgather(m2ct_q[q], m2tf_q[q])
                        load_q(m2tq3[q], m2tf_q[q])

                    # ---- staged pipeline ----
                    for h in range(NQ):
                        p1_chunk(h)
                        mc_batch(h)
                        if 1 <= h <= 2:
                            layer_m(mt_at, xhi, xlo, lambda m: onesr[0:1, :],
                                    l1_close, psl1p, range(4 * (h - 1), 4 * h))
                    # P2 q0/q1 fill the PE hole while the q2/q3 loads land
                    p2_q(0)
                    p2_q(1)
                    layer_m(mt_at, xhi, xlo, lambda m: onesr[0:1, :],
                            l1_close, psl1p, range(8, 12))
                    p2_q(2)
                    layer_m(mt_at, xhi, xlo, lambda m: onesr[0:1, :],
                            l1_close, psl1p, range(12, 16))
                    p2_q(NQ - 1)

                # ---- L2 and A2: rotating 8-bank pool, long clean bursts ----
                with (
                    tc.tile_pool(name="psl", bufs=8, space="PSUM") as pslp,
                    tc.tile_pool(name="xo", bufs=4) as xop,
                ):
                    def out_close(ps, m):
                        xo = xop.tile([128, NB], BF16, tag="xo", name="xo")
                        if m % 2 == 0:
                            nc.scalar.mul(xo[:, :], ps[:, 0:NB], UN)
                        else:
                            nc.vector.tensor_scalar_mul(xo[:, :], ps[:, 0:NB], UN)
                        nc.sync.dma_start(
                            out=out_d[128 * m:128 * (m + 1), :], in_=xo[:, :])

                    def layer_m2(lhs_at, rh3, rl3, bias_row, close, ms):
                        for m in ms:
                            ps = pslp.tile([128, 512], F32, tag="psl", name="psl")
                            for r in range(RB):
                                for rhs3 in ([rh3] if rl3 is None else [rh3, rl3]):
                                    nc.tensor.matmul(
                                        ps[:, 0:NB],
                                        lhs_at(r, m),
                                        rhs3[:, 2 * r:2 * r + 2, :],
                                        start=(r == 0 and rhs3 is rh3), stop=False,
                                        perf_mode=DR, skip_group_check=True,
                                    )
                            bias_mm(ps, bias_row(m))
                            close(ps, m)

                    layer_m2(mt_at, x1h3, None, lambda m: onesr[0:1, :],
                             l2_close, range(KT))
                    layer_m2(m2t_at, x2h3, None,
                             lambda m: s1a2[0:1, 128 * m:128 * (m + 1)],
                             out_close, range(KT))

    nc.compile()
    return nc


def make_in_maps(x, rows, cols, adj_vals, w_vals, bias, dt: str = DEFAULT_DT):
    """Host-side scatter + fp8 casts/splits + shard. In-maps for cores 0..7."""
    import ml_dtypes
    E4 = ml_dtypes.float8_e4m3

    rows = np.asarray(rows).astype(np.int64)
    cols = np.asarray(cols).astype(np.int64)
    adj_vals = np.asarray(adj_vals, dtype=np.float64)
    w_vals = np.asarray(w_vals, dtype=np.float64)
    x = np.asarray(x, dtype=np.float32)
    bias = np.asarray(bias, dtype=np.float32)

    # AT[c, r] = A[r, c] (dense transpose of the scattered COO)
    at = np.bincount(cols * N + rows, weights=adj_vals, minlength=N * N).reshape(N, N)
    w = np.bincount(rows * N + cols, weights=w_vals, minlength=N * N).reshape(N, N)

    at8 = np.ascontiguousarray(at.astype(np.float32)).astype(E4)
    w8 = w.astype(np.float32).astype(E4)
    xh = x.astype(E4)
    xl = (x - xh.astype(np.float32)).astype(E4)

    onesr = np.ones((1, 128), dtype=ml_dtypes.bfloat16)
    # s = A @ (W @ 1) + 1 straight from the COO (exact, O(nnz))
    w1 = np.bincount(rows, weights=w_vals, minlength=N)
    s1 = np.bincount(rows, weights=adj_vals * w1[cols], minlength=N) + 1.0
    s1r = np.ascontiguousarray(
        (s1 * SQ * SX2).astype(ml_dtypes.bfloat16)[None, :])

    in_maps = []
    for c in range(N_CORES):
        sl = slice(c * NB, (c + 1) * NB)
        # wc: [128, (k c)] partition-major;  xhl: [128, (k h c)]
        wcr = np.ascontiguousarray(
            w8[:, sl].reshape(KT, 128, NB).transpose(1, 0, 2).reshape(128, KT * NB))
        xhl = np.stack([xh[:, sl], xl[:, sl]], axis=1)  # [N, 2, NB]
        xhlr = np.ascontiguousarray(
            xhl.reshape(KT, 128, 2, NB).transpose(1, 0, 2, 3).reshape(128, KT * 2 * NB))
        in_maps.append({
            "at": at8,
            "wc": wcr,
            "xhl": xhlr,
            "biasc": np.ascontiguousarray(
                bias[sl].astype(ml_dtypes.bfloat16)[None, :]),
            "onesr": onesr,
            "s1r": s1r,
        })
    return in_maps


_NC_CACHE = {}


def kernel(x, rows, cols, adj_vals, w_vals, bias):
    if "nc" not in _NC_CACHE:
        _NC_CACHE["nc"] = build_nc(iters=1)
    nc = _NC_CACHE["nc"]
    in_maps = make_in_maps(x, rows, cols, adj_vals, w_vals, bias)
    for attempt in range(2):
        res = run_bass_kernel_spmd(nc, in_maps, core_ids=list(range(N_CORES)))
        out = np.empty((N, N), dtype=np.float32)
        for c in range(N_CORES):
            out[:, c * NB:(c + 1) * NB] = \
                res.results[c]["out"].astype(np.float32)
        # guard against rare backend transients: retry on non-finite output
        if np.isfinite(out).all():
            break
    return out



# revision 24
# speedup vs baseline: 1.1364x; 1.1364x over previous
"""Trainium2 Bass kernel for AdultConnectomeNetwork (gnn_message_passing).

Reference computation:
    A = scatter(rows, cols, adj_vals)   # [N, N] dense from COO, dups sum
    W = scatter(rows, cols, w_vals)     # [N, N]
    M = A @ W
    for _ in range(4): x = M @ x + bias[None, :]

Structure (plain 4-layer; the baseline's M^2 route costs an extra
AllGather round trip for zero PE savings, so it's gone):
    P1   per core: McT block = Wc^T @ A^T            (column shard of M)
    AG   AllGather MT (one matrix, quarter-pipelined: [256,512] slices
         -> [2048,512] -> reload as [128,16,512] lhsT tiles)
    L1   X1 = M X + 1 b^T      (X in split hi+lo fp8)
    L2-4 X_l = M X_{l-1}       (single fp8 plane, power-of-2 scales;
         the bias add is dropped where |b|/|X_l| < 1e-4 — X2 ~ 2e4,
         X3 ~ 2e8, X4 ~ 2e10 vs b ~ 1)

Precision: fp8 e4m3 matmuls in DoubleRow perf mode (0.5 cyc/row in the
cost model). M single fp8; x hi+lo fp8 (first-layer input error is not
sqrt(N)-suppressed, later layers' is); X2/X3 stored fp8 with 2^-10 /
2^-20 scales; output bf16. L1's bias rides the PSUM->SBUF close as one
DVE scalar_tensor_tensor (out = P*1 + b128) — no PE bias matmuls (a K=1
bias matmul costs a full free-dim's cycles in the cost model).

Scheduling (cost-model driven):
  - DMA order: wc, at quarters (SP queue) first; xhl/bias ride behind;
    stage/AG/load chains slot into the stream by ready-time as P1 chunks
    complete. Everything before the last gathered quarter is critical.
  - PE: warm-up matmuls (garbage fp8 into a junk PSUM bank) keep the
    p-state ramp at 2.4 GHz through the DMA-bound head; L1 quarters run
    in the shadow of the gather; partially-emitted L2 groups (k-pairs
    0-5 of m-tiles 0-3) fill the PE hole while the last quarter lands;
    L2-4 then run back-to-back from SBUF.
  - closes alternate ACT/DVE so the PSUM drain keeps up with the PE.
"""

import numpy as np

import concourse.bass as bass
import concourse.mybir as mybir
from concourse import bacc, tile
from concourse.bass_utils import run_bass_kernel_spmd

N = 2048
NNZ = 131072
LAYERS = 4
N_CORES = 8
NB = N // N_CORES          # 256 columns of x per core
KT = N // 128              # 16 k-tiles
KP = KT // 2               # 8 DoubleRow k-pairs
NQ = 4                     # 512-column quarters
Q = N // NQ                # 512

S2 = 2.0 ** -10            # scale on stored X2 (max |X2| ~1.8e5 -> ~173)
S3 = 2.0 ** -20            # scale on stored X3 (max |X3| ~1.8e8 -> ~172)
WARM0 = 30                 # warm-up matmuls before P1 chunk 0
WARMI = 6                  # warm-up matmuls between P1 chunks
WARMG = 80                 # warm-up matmuls in the post-P1 gather hole
WARMF = 0                 # warm-up matmuls after each L2 feed stage

DEFAULT_DT = "fp8"
DR = mybir.MatmulPerfMode.DoubleRow
F32 = mybir.dt.float32
BF16 = mybir.dt.bfloat16
FP8 = mybir.dt.float8e4
ADD = mybir.AluOpType.add
MULT = mybir.AluOpType.mult


def build_nc(iters: int = 1, sim_single_core: bool = False, dt: str = DEFAULT_DT) -> bacc.Bacc:
    """sim_single_core: replace each AllGather with a broadcast copy so the
    graph is collective-free (runnable under TimelineSim) while keeping the
    same dependency structure + data volume (bandwidth-honest stand-in).
    That variant is NOT functionally correct."""
    nc = bacc.Bacc("TRN2", target_bir_lowering=False, num_devices=N_CORES)

    at_d = nc.dram_tensor("at", [N, N], FP8, kind="ExternalInput")
    # pre-arranged on host to SBUF partition-major layouts (contiguous loads)
    wc_d = nc.dram_tensor("wc", [128, KT * NB], FP8, kind="ExternalInput")
    xhl_d = nc.dram_tensor("xhl", [128, KT * 2 * NB], FP8, kind="ExternalInput")
    biasr_d = nc.dram_tensor("biasr", [1, NB], BF16, kind="ExternalInput")
    out_d = nc.dram_tensor("out", [N, NB], BF16, kind="ExternalOutput")

    with tile.TileContext(nc) as tc:
        with (
            tc.tile_pool(name="const", bufs=1) as constp,
            tc.tile_pool(name="x", bufs=1) as xp,
            tc.tile_pool(name="dram", bufs=1, space="DRAM") as dram,
        ):
            b128 = constp.tile([128, NB], BF16, tag="b128")

            for it in range(iters):
                ag_as = "Local" if sim_single_core else "Shared"
                mct_q = [dram.tile([NB, Q], FP8, name=f"mct{q}_{it}")
                         for q in range(NQ)]
                mtf_q = [dram.tile([N, Q], FP8, name=f"mtf{q}_{it}",
                                   addr_space=ag_as) for q in range(NQ)]

                wc_sb = xp.tile([128, KT * NB], FP8, tag="wc", name=f"wc_{it}")
                wc3 = wc_sb[:, :].rearrange("p (k c) -> p k c", k=KT)
                xhl_sb = xp.tile([128, KT * 2 * NB], FP8, tag="xhl", name=f"xhl_{it}")
                xhl4 = xhl_sb[:, :].rearrange("p (k h c) -> p k h c", k=KT, h=2)
                mct_sb = xp.tile([128, 2 * N], FP8, tag="mct", name=f"mct_{it}")
                mct3 = mct_sb[:, :].rearrange("p (m j) -> p m j", m=2)
                x1_sb = xp.tile([128, KT * NB], FP8, tag="x1", name=f"x1_{it}")
                x13 = x1_sb[:, :].rearrange("p (k c) -> p k c", k=KT)
                x2_sb = xp.tile([128, KT * NB], FP8, tag="x2", name=f"x2_{it}")
                x23 = x2_sb[:, :].rearrange("p (k c) -> p k c", k=KT)
                x3_sb = xp.tile([128, KT * NB], FP8, tag="x3", name=f"x3_{it}")
                x33 = x3_sb[:, :].rearrange("p (k c) -> p k c", k=KT)
                mtq = [xp.tile([128, KT * Q], FP8, tag=f"mtq{q}",
                               name=f"mtq{q}_{it}") for q in range(NQ)]
                mtq3 = [t[:, :].rearrange("p (k j) -> p k j", k=KT) for t in mtq]

                def allgather(src_d, dst_d):
                    if sim_single_core:
                        # one broadcast: the 8 rank-slices as a stride-0
                        # source repeat (same bytes/descriptors as 8 copies)
                        sap = src_d[:, :]
                        rep = bass.AP(sap.tensor, sap.offset,
                                      [[0, N_CORES]] + list(sap.ap))
                        nc.gpsimd.dma_start(
                            out=dst_d[:, :].rearrange("(r p) j -> r p j",
                                                      r=N_CORES),
                            in_=rep)
                    else:
                        nc.gpsimd.collective_compute(
                            "AllGather", mybir.AluOpType.bypass,
                            replica_groups=[list(range(N_CORES))],
                            ins=[src_d.opt()], outs=[dst_d.opt()],
                        )

                def load_q(dst3, src_d):
                    # whole gathered quarter in one DMA (2048-row source)
                    nc.scalar.dma_start(
                        out=dst3[:, :, :],
                        in_=src_d[:, :].rearrange("(k p) j -> p k j", p=128),
                    )

                with (
                    tc.tile_pool(name="at", bufs=1) as atp,
                    tc.tile_pool(name="ps", bufs=8, space="PSUM") as psp,
                    tc.tile_pool(name="xo", bufs=3) as xop,
                ):
                    at_t = [atp.tile([128, KT, Q], FP8, tag=f"at{h}",
                                     name=f"at{h}_{it}") for h in range(NQ)]

                    # ---- SP-queue DMAs, in critical-path order; at2/at3 are
                    # issued late from the DVE queue (inside p1_chunk) so the
                    # stage0/AG0/load0 chain wins the DMA ready-time FIFO and
                    # L1 can start in the shadow of the rest of the gather ----
                    def at_src(h):
                        return at_d[:, Q * h:Q * (h + 1)] \
                            .rearrange("(k p) j -> p k j", p=128)

                    nc.sync.dma_start(out=wc_sb[:, :], in_=wc_d[:, :])
                    nc.sync.dma_start(out=at_t[0][:, 0:4, :], in_=at_src(0)[:, 0:4, :])
                    nc.sync.dma_start(out=at_t[0][:, 4:, :], in_=at_src(0)[:, 4:, :])
                    nc.sync.dma_start(out=at_t[1][:, :, :], in_=at_src(1))
                    nc.sync.dma_start(out=xhl_sb[:, :], in_=xhl_d[:, :])
                    nc.sync.dma_start(out=b128[:, :], in_=bass.AP(
                        biasr_d[0:1, :].tensor, biasr_d[0:1, :].offset,
                        [[0, 128]] + list(biasr_d[0:1, :].ap)[1:]))

                    def warm(n, src3=None):
                        # garbage fp8 matmuls into a junk PSUM tile: keeps the
                        # PE p-state ramp warm through DMA waits (never read).
                        # src3 gates the burst on a tile landing, so the fill
                        # sits at the END of a known PE hole, not its start.
                        s = mct3 if src3 is None else src3
                        for _ in range(n):
                            psw = psp.tile([128, 512], F32, tag="ps", name="psw")
                            nc.tensor.matmul(
                                psw[:, :],
                                s[:, 0:2, 0:128], s[:, 0:2, 0:512],
                                start=True, stop=True,
                                perf_mode=DR, skip_group_check=True,
                            )

                    def p1_chunk(h):
                        for mi in range(2):
                            ps = psp.tile([128, 512], F32, tag="ps", name="ps1")
                            for kp in range(KP):
                                nc.tensor.matmul(
                                    ps[:, :],
                                    wc3[:, 2 * kp:2 * kp + 2, 128 * mi:128 * (mi + 1)],
                                    at_t[h][:, 2 * kp:2 * kp + 2, :],
                                    start=(kp == 0), stop=(kp == KP - 1),
                                    perf_mode=DR, skip_group_check=True,
                                )
                            dst = mct3[:, mi, Q * h:Q * (h + 1)]
                            if mi == 0:
                                nc.scalar.copy(dst, ps[:, :])
                            else:
                                nc.vector.tensor_copy(dst, ps[:, :])
                        nc.scalar.dma_start(
                            out=mct_q[h][:, :].rearrange("(m p) j -> p m j", p=128),
                            in_=mct3[:, :, Q * h:Q * (h + 1)],
                        )
                        allgather(mct_q[h], mtf_q[h])
                        load_q(mtq3[h], mtf_q[h])
                        if 1 <= h <= 2:
                            # late at quarter loads: Pool-queued behind AG_h's
                            # descriptor gen, so their transfers rank behind
                            # the stage0/AG0/load0 chain in the DMA FIFO
                            nc.gpsimd.dma_start(out=at_t[h + 1][:, :, :],
                                                in_=at_src(h + 1))

                    def mt_at(r, m):
                        return mtq3[m // 4][:, 2 * r:2 * r + 2,
                                           128 * (m % 4):128 * (m % 4 + 1)]

                    # L1 close: out = PSUM + b128 (one DVE op, bias matters
                    # at X1 scale). L2-4 closes: pure scale, ACT/DVE alternate.
                    def l1_close(ps, m):
                        nc.vector.scalar_tensor_tensor(
                            x13[:, m, :], ps[:, 0:NB], 1.0, b128[:, :],
                            op0=MULT, op1=ADD)

                    def mul_close(dst3, scale):
                        def _c(ps, m):
                            if m % 2 == 0:
                                nc.scalar.mul(dst3[:, m, :], ps[:, 0:NB], scale)
                            else:
                                nc.vector.tensor_scalar_mul(
                                    dst3[:, m, :], ps[:, 0:NB], scale)
                        return _c

                    def emit_group(ps, lhs_at, rhs_list_of, m, s_lo, s_hi,
                                   start, stop):
                        rhss = [(r, rhs) for r in range(s_lo, s_hi)
                                for rhs in rhs_list_of(r)]
                        for i, (r, rhs) in enumerate(rhss):
                            nc.tensor.matmul(
                                ps[:, 0:NB], lhs_at(r, m), rhs,
                                start=(start and i == 0),
                                stop=(stop and i == len(rhss) - 1),
                                perf_mode=DR, skip_group_check=True,
                            )

                    # ---- head: warm-up + P1 quarters feeding the gather ----
                    warm(WARM0)
                    for h in range(NQ):
                        p1_chunk(h)
                        if h < NQ - 1:
                            warm(WARMI)
                    # fill the post-P1 hole (PE waits ~9us for the first
                    # gathered quarter): keeps the p-state ramp at full so L1
                    # starts at 2.4 GHz
                    warm(WARMG)

                    xhi = xhl4[:, :, 0, :]
                    xlo = xhl4[:, :, 1, :]
                    l1_rhs = lambda r: [xhi[:, 2 * r:2 * r + 2, :],
                                        xlo[:, 2 * r:2 * r + 2, :]]
                    x1_rhs = lambda r: [x13[:, 2 * r:2 * r + 2, :]]
                    x2_rhs = lambda r: [x23[:, 2 * r:2 * r + 2, :]]
                    x3_rhs = lambda r: [x33[:, 2 * r:2 * r + 2, :]]

                    # L1 runs in quarter blocks as gathered quarters land;
                    # warm-ups bridge the inter-load PE holes so the ramp
                    # stays at full clock into the tail
                    for blk in range(4):
                        for m in range(4 * blk, 4 * blk + 4):
                            ps = psp.tile([128, 512], F32, tag="ps", name=f"l1_{m}")
                            emit_group(ps, mt_at, l1_rhs, m, 0, KP, True, True)
                            l1_close(ps, m)
                        if blk < 3:
                            warm(WARMF)

                    # ---- tail: L2, L3, L4 back-to-back ----
                    l2_close = mul_close(x23, float(S2))
                    for m in range(KT):
                        ps = psp.tile([128, 512], F32, tag="ps", name=f"l2_{m}")
                        emit_group(ps, mt_at, x1_rhs, m, 0, KP, True, True)
                        l2_close(ps, m)

                    l3_close = mul_close(x33, float(S3 / S2))
                    for m in range(KT):
                        ps = psp.tile([128, 512], F32, tag="ps", name=f"l3_{m}")
                        emit_group(ps, mt_at, x2_rhs, m, 0, KP, True, True)
                        l3_close(ps, m)

                    # L4: closes land in a 4-tile staging buffer; one DMA per
                    # 4 m-tiles (16 singleton DMAs serialize ~625ns each on
                    # the single-slot HWDGE right at the kernel tail)
                    xo4 = None
                    for m in range(KT):
                        ps = psp.tile([128, 512], F32, tag="ps", name=f"l4_{m}")
                        emit_group(ps, mt_at, x3_rhs, m, 0, KP, True, True)
                        if m % 4 == 0:
                            xo4 = xop.tile([128, 4 * NB], BF16, tag="xo", name="xo")
                            xo4v = xo4[:, :].rearrange("p (s c) -> p s c", s=4)
                        if m % 2 == 0:
                            nc.scalar.mul(xo4v[:, m % 4, :], ps[:, 0:NB],
                                          float(1.0 / S3))
                        else:
                            nc.vector.tensor_scalar_mul(
                                xo4v[:, m % 4, :], ps[:, 0:NB], float(1.0 / S3))
                        if m % 4 == 3:
                            q4 = m // 4
                            nc.sync.dma_start(
                                out=out_d[512 * q4:512 * (q4 + 1), :]
                                .rearrange("(s p) c -> p s c", p=128),
                                in_=xo4v[:, :, :])

    nc.compile()
    return nc


def make_in_maps(x, rows, cols, adj_vals, w_vals, bias, dt: str = DEFAULT_DT):
    """Host-side scatter + fp8 casts/splits + shard. In-maps for cores 0..7."""
    import ml_dtypes
    E4 = ml_dtypes.float8_e4m3

    rows = np.asarray(rows).astype(np.int64)
    cols = np.asarray(cols).astype(np.int64)
    adj_vals = np.asarray(adj_vals, dtype=np.float64)
    w_vals = np.asarray(w_vals, dtype=np.float64)
    x = np.asarray(x, dtype=np.float32)
    bias = np.asarray(bias, dtype=np.float32)

    # AT[c, r] = A[r, c] (dense transpose of the scattered COO)
    at = np.bincount(cols * N + rows, weights=adj_vals, minlength=N * N).reshape(N, N)
    w = np.bincount(rows * N + cols, weights=w_vals, minlength=N * N).reshape(N, N)

    at8 = np.ascontiguousarray(at.astype(np.float32)).astype(E4)
    w8 = w.astype(np.float32).astype(E4)
    xh = x.astype(E4)
    xl = (x - xh.astype(np.float32)).astype(E4)

    in_maps = []
    for c in range(N_CORES):
        sl = slice(c * NB, (c + 1) * NB)
        # wc: [128, (k c)] partition-major;  xhl: [128, (k h c)]
        wcr = np.ascontiguousarray(
            w8[:, sl].reshape(KT, 128, NB).transpose(1, 0, 2).reshape(128, KT * NB))
        xhl = np.stack([xh[:, sl], xl[:, sl]], axis=1)  # [N, 2, NB]
        xhlr = np.ascontiguousarray(
            xhl.reshape(KT, 128, 2, NB).transpose(1, 0, 2, 3).reshape(128, KT * 2 * NB))
        in_maps.append({
            "at": at8,
            "wc": wcr,
            "xhl": xhlr,
            "biasr": np.ascontiguousarray(
                bias[sl].astype(ml_dtypes.bfloat16)[None, :]),
        })
    return in_maps


_NC_CACHE = {}


def kernel(x, rows, cols, adj_vals, w_vals, bias):
    if "nc" not in _NC_CACHE:
        _NC_CACHE["nc"] = build_nc(iters=1)
    nc = _NC_CACHE["nc"]
    in_maps = make_in_maps(x, rows, cols, adj_vals, w_vals, bias)
    for attempt in range(2):
        res = run_bass_kernel_spmd(nc, in_maps, core_ids=list(range(N_CORES)))
        out = np.empty((N, N), dtype=np.float32)
        for c in range(N_CORES):
            out[:, c * NB:(c + 1) * NB] = \
                res.results[c]["out"].astype(np.float32)
        # guard against rare backend transients: retry on non-finite output
        if np.isfinite(out).all():
            break
    return out
